# revision 12
# baseline (speedup 1.0000x reference)
"""CapsNet dynamic-routing kernel for Trainium2, 8 NeuronCores.

Problem: nn_Caps_47742856462336
  u:    [32, 1152, 16] f32
  W:    [1, 32, 1152, 32, 16] f32
  bias: [1, 32, 32] f32
  out = 2-iter dynamic routing -> [32, 32, 32] f32

Sharding: tensor-parallel over in_caps (k): 1152/8 = 144 per core. Each core
holds its W k-shard resident in SBUF (two bf16 layouts, host-prepared), does
all contractions on the PE, and the routing state is combined with two tiny
(131 KB) AllReduces. All cores end with the identical output.

Per core (B=32 batch, J=32 out_caps, O=32 out_dim, I=16 in_dim, KL=144 local
in_caps; j = 4*j8 + j4; k main 0..128 on partitions, tail k16=k-128 packed as
(ih,k16) partitions with il accumulated, i = 2*ih + il):
  s0   = sum_{k,i} u*W               (PE, 36 matmuls of 512)       -> AR
  v0   = squash(s0/32 + bias)        (layout [(q,b), 256], q=j8//2)
  Wv   = sum_o v0*W                  (PE j4-blockdiag; PSUM chunks)
  A    = sum_i u*Wv                  (ACT evacuates PSUM, DVE mul+tree)
  c1   = softmax_j(A)                (no max-sub; exp on ACT)
  s1   = sum_{k,i} (c1*u)*W          (PE, per-j8; 18 accums of 128)  -> AR
  out  = squash(s1 + bias)
"""

import os
import sys
import numpy as np

for _p in ("/opt/trn_rl_repo", os.path.expanduser("~/.axon_site/_ro/trn_rl_repo")):
    if os.path.isdir(_p) and _p not in sys.path:
        sys.path.insert(0, _p)

import ml_dtypes  # noqa: E402

BF = ml_dtypes.bfloat16

B = 32      # batch
J = 32      # out_caps
O = 32      # out_dim
I = 16      # in_dim
KG = 1152   # global in_caps
NC = 8      # cores
KL = KG // NC   # 144 in_caps per core
KT1 = 128       # k main tile (k on partitions)
KT2 = KL - KT1  # 16 ragged k, packed as (ih, k16) partitions, il accumulated
EPS = 1e-7

J8 = 8   # j // 4
J4 = 4   # j %  4
JO = J * O            # 1024
KI = KL * I           # 2304
FW = J8 * KI          # 18432 (wo free size)

JUNK_PRE = 16    # PE warmup matmuls of 512 before s0 (ramp p-state)
JUNK_AR1 = 56    # PE keep-hot matmuls of 512 during AllReduce 1


# ---------------------------------------------------------------------------
# host-side data prep: per-core DMA-friendly bf16/f32 layouts
# ---------------------------------------------------------------------------

def host_prep(u, W, bias):
    """Returns list of 8 dicts of named np arrays (the per-core DRAM inputs)."""
    u = np.asarray(u, dtype=np.float32)
    W = np.asarray(W, dtype=np.float32)
    bias = np.asarray(bias, dtype=np.float32)
    Wf = W[0]                      # [J, KG, O, I]
    biasf = bias[0]                # [J, O]

    # bias0q [(q,b), (j8', j4, o)] : j8 = 2q + j8'
    bq = biasf.reshape(J8, J4, O).reshape(4, 2 * J4 * O)
    bias0q = np.broadcast_to(bq.reshape(4, 1, 256), (4, B, 256))
    bias0q = np.ascontiguousarray(bias0q.reshape(128, 256), dtype=np.float32)
    # bias1 [(j4,b), (j8,o)]
    b1 = biasf.reshape(J8, J4, O).transpose(1, 0, 2)          # [j4, j8, o]
    b1 = np.broadcast_to(b1.reshape(J4, 1, J8 * O), (J4, B, J8 * O))
    bias1 = np.ascontiguousarray(b1.reshape(J4 * B, J8 * O), dtype=np.float32)

    ins = []
    for c in range(NC):
        ks = c * KL
        Wc = Wf[:, ks:ks + KL]                 # [J, KL, O, I]
        uc = u[:, ks:ks + KL]                  # [B, KL, I]
        Wr = Wc.reshape(J8, J4, KL, O, I)      # j8 j4 k o i

        # w1a [128=k, (i, j8, j4, o)]
        w1a = Wr[:, :, :KT1].transpose(2, 4, 0, 1, 3).reshape(KT1, I * JO)
        # w1bp [128=(ih,k16), (il, j8, j4, o)], i = 2*ih + il
        wt = Wr[:, :, KT1:].reshape(J8, J4, KT2, O, 8, 2)  # j8 j4 k16 o ih il
        w1bp = wt.transpose(4, 2, 5, 0, 1, 3).reshape(128, 2 * JO)
        # wo [128=(j4,o), (j8, k, i)]
        wo = Wr.transpose(1, 3, 0, 2, 4).reshape(J4 * O, J8 * KL * I)
        # u1a [128=k, (i, b)]
        u1a = uc[:, :KT1].transpose(1, 2, 0).reshape(KT1, I * B)
        # u1bp [128=(ih,k16), (il, b)]
        ut = uc[:, KT1:].reshape(B, KT2, 8, 2)             # b k16 ih il
        u1bp = ut.transpose(2, 1, 3, 0).reshape(128, 2 * B)
        # urep [128=(j4,b), (k, i)]
        urep = np.broadcast_to(uc.reshape(1, B, KI), (J4, B, KI))
        urep = urep.reshape(J4 * B, KI)

        ins.append({
            "w1a": np.ascontiguousarray(w1a).astype(BF),
            "w1bp": np.ascontiguousarray(w1bp).astype(BF),
            "wo": np.ascontiguousarray(wo).astype(BF),
            "u1a": np.ascontiguousarray(u1a).astype(BF),
            "u1bp": np.ascontiguousarray(u1bp).astype(BF),
            "urep": np.ascontiguousarray(urep).astype(BF),
            "bias0q": bias0q,
            "bias1": bias1,
        })
    return ins


def host_unpack(out):
    """out [(j4,b), (j8,o)] f32 -> [B, J, O] with j = 4*j8 + j4."""
    return np.ascontiguousarray(
        out.reshape(J4, B, J8, O).transpose(1, 2, 0, 3).reshape(B, J, O)
    )


# ---------------------------------------------------------------------------
# device program
# ---------------------------------------------------------------------------

def build_program(tc, outs, ins, n_cores=NC, use_cc=True, stop_after=None):
    import concourse.bass as bass
    from concourse import mybir, masks

    F32 = mybir.dt.float32
    BF16 = mybir.dt.bfloat16
    ADD = mybir.AluOpType.add
    MULT = mybir.AluOpType.mult
    AX = mybir.AxisListType.X
    ACT = mybir.ActivationFunctionType

    nc = tc.nc
    w1a_d = ins["w1a"]; w1bp_d = ins["w1bp"]; wo_d = ins["wo"]
    u1a_d = ins["u1a"]; u1bp_d = ins["u1bp"]; urep_d = ins["urep"]
    bias0q_d = ins["bias0q"]; bias1_d = ins["bias1"]
    out_d = outs["out"]

    import contextlib
    stack = contextlib.ExitStack()
    with stack:
        pool = stack.enter_context(tc.tile_pool(name="main", bufs=1))
        psum = stack.enter_context(tc.tile_pool(name="psum", bufs=1, space="PSUM"))
        dram = stack.enter_context(tc.tile_pool(name="dram", bufs=1, space="DRAM"))

        # ---- resident inputs -------------------------------------------------
        w1a = pool.tile([KT1, I * JO], BF16)
        w1bp = pool.tile([128, 2 * JO], BF16)
        wo = pool.tile([128, FW], BF16)
        u1a = pool.tile([KT1, I * B], BF16)
        u1bp = pool.tile([128, 2 * B], BF16)
        urep = pool.tile([128, KI], BF16)
        bias0q = pool.tile([128, 256], F32)
        bias1 = pool.tile([128, J8 * O], F32)
        ident = pool.tile([128, 128], BF16)
        junk = pool.tile([128, 512], BF16)

        # DMA order = priority: s0 inputs first (w1a in chunks so s0 can chase
        # the DMA), then the tail, then wo per-j8 (Wv chases), then the rest.
        nc.sync.dma_start(u1a[:], u1a_d)
        nc.sync.dma_start(u1bp[:], u1bp_d)
        w1av = w1a[:].rearrange("k (i f) -> k i f", i=I)
        w1ad = w1a_d.rearrange("k (i f) -> k i f", i=I)
        for ch in range(8):
            nc.sync.dma_start(w1av[:, 2 * ch:2 * ch + 2], w1ad[:, 2 * ch:2 * ch + 2])
        nc.sync.dma_start(w1bp[:], w1bp_d)
        wov = wo[:].rearrange("p (j8 f) -> p j8 f", j8=J8)
        wod = wo_d.rearrange("p (j8 f) -> p j8 f", j8=J8)
        for j8 in range(J8):
            nc.sync.dma_start(wov[:, j8], wod[:, j8])
        for tile_, dram_ in ((urep, urep_d), (bias0q, bias0q_d), (bias1, bias1_d)):
            nc.sync.dma_start(tile_[:], dram_)
        masks.make_identity(nc, ident[:])
        nc.vector.memset(junk[:], 0.0)

        # ---- collective bounce buffers --------------------------------------
        cc0_in = dram.tile([128, 256], F32)
        cc0_out = dram.tile([128, 256], F32)
        cc1_in = dram.tile([128, J8 * O], F32)
        cc1_out = dram.tile([128, J8 * O], F32)
        rg = [list(range(n_cores))]

        def _finish(tile_ap, rows):
            """Timing-bisect helper: route a dependency on `tile_ap` to out."""
            z = pool.tile([128, J8 * O], F32, tag="finz")
            nc.vector.memset(z[:], 0.0)
            nc.vector.tensor_copy(z[:rows, :1], tile_ap[:rows, :1])
            nc.scalar.dma_start(out_d, z[:])

        # ---- PE warmup (ramp p-state during input DMA) ----------------------
        def junk_mm(n, tag):
            for t in range(n):
                jp = psum.tile([128, 512], F32, tag="wv", bufs=2,
                               name=f"jp_{tag}_{t}")
                nc.tensor.matmul(jp[:], junk[:, :128], junk[:],
                                 start=True, stop=True, skip_group_check=True)

        junk_mm(JUNK_PRE, "pre")

        # ---- epsilon + ACT table preload (Sqrt) ------------------------------
        epsb = pool.tile([128, 1], F32)
        nc.vector.memset(epsb[:], EPS)
        actwarm = pool.tile([128, 1], F32)
        nc.vector.memset(actwarm[:], 1.0)
        nc.scalar.activation(actwarm[:], actwarm[:], ACT.Sqrt, bias=epsb[:])

        # ---- s0 partial = sum_{k,i} u*W  ->  [32, (j8,j4,o)] -----------------
        ps0full = psum.tile([128, JO], F32, tag="acc")
        ps0 = ps0full[:B]
        u1av = u1a[:].rearrange("k (i b) -> k i b", i=I)
        u1bpv = u1bp[:].rearrange("p (il b) -> p il b", il=2)
        w1bpv = w1bp[:].rearrange("p (il f) -> p il f", il=2)
        for h in range(2):
            sl = slice(512 * h, 512 * h + 512)
            for i in range(I):
                nc.tensor.matmul(ps0[:, sl], u1av[:, i], w1av[:, i].rearrange(
                    "k (x f) -> k x f", x=2)[:, h], start=(i == 0), stop=False)
            for il in range(2):
                nc.tensor.matmul(ps0[:, sl], u1bpv[:, il],
                                 w1bpv[:, il, sl], start=False, stop=(il == 1))

        # evacuate into [(q,b), 256] layout (4 partition-shifted copies)
        s0q = pool.tile([128, 256], F32)
        for q in range(4):
            nc.vector.tensor_copy(s0q[32 * q:32 * q + 32, :],
                                  ps0[:, 256 * q:256 * q + 256])
        if stop_after == "s0":
            return _finish(s0q[:], 128)

        junk_mm(JUNK_AR1, "ar1")

        if use_cc:
            nc.gpsimd.dma_start(cc0_in[:], s0q[:])
            nc.gpsimd.collective_compute(
                "AllReduce", ADD, replica_groups=rg,
                ins=[cc0_in.opt()], outs=[cc0_out.opt()])
            s0g = pool.tile([128, 256], F32)
            nc.gpsimd.dma_start(s0g[:], cc0_out[:])
        else:
            s0g = s0q

        # ---- v0 = squash(s0/32 + bias) --------------------------------------
        def squash(dst, src, nj):
            """dst[128, nj*O] = squash over o of src (layout [.., (j, o)])."""
            t = pool.tile([128, nj * O], F32, tag="sqt")
            nc.vector.tensor_mul(t[:], src, src)
            sq = pool.tile([128, nj], F32, tag="sqsq")
            nc.vector.tensor_reduce(
                sq[:], t[:].rearrange("p (j o) -> p j o", o=O), axis=AX, op=ADD)
            one = pool.tile([128, nj], F32, tag="sqone")
            nc.vector.tensor_scalar_add(one[:], sq[:], 1.0)
            r1 = pool.tile([128, nj], F32, tag="sqr1")
            nc.vector.reciprocal(r1[:], one[:])
            rt = pool.tile([128, nj], F32, tag="sqrt")
            nc.scalar.activation(rt[:], sq[:], ACT.Sqrt, bias=epsb[:])
            m = pool.tile([128, nj], F32, tag="sqm")
            nc.vector.tensor_mul(m[:], r1[:], rt[:])
            mv = m[:].unsqueeze(2).broadcast_to((128, nj, O))
            nc.vector.tensor_mul(
                dst.rearrange("p (j o) -> p j o", o=O),
                src.rearrange("p (j o) -> p j o", o=O), mv)

        s0f = pool.tile([128, 256], F32)
        nc.vector.scalar_tensor_tensor(
            s0f[:], s0g[:], 1.0 / 32.0, bias0q[:], op0=MULT, op1=ADD)
        v0b = pool.tile([128, 256], BF16)
        squash(v0b[:], s0f[:], 8)
        if stop_after == "v0":
            return _finish(v0b[:], 128)

        # ---- vst[(j4,o), (j8,b)] = transposed v0 blocks; v0bd blockdiag -----
        v0b32 = pool.tile([B, JO], BF16)
        for q in range(4):
            nc.vector.tensor_copy(v0b32[:, 256 * q:256 * q + 256],
                                  v0b[32 * q:32 * q + 32, :])
        vst = pool.tile([128, J8 * B], BF16)
        for j8 in range(J8):
            pt = psum.tile([128, 128], BF16, tag="tr", bufs=2)
            nc.tensor.matmul(pt[:, :B], v0b32[:, 128 * j8:128 * j8 + 128],
                             ident[:B, :B], is_transpose=True)
            nc.vector.tensor_copy(vst[:, B * j8:B * j8 + B], pt[:, :B])
        v0bd = pool.tile([128, J8 * 128], BF16)
        nc.vector.memset(v0bd[:], 0.0)
        v0bdv = v0bd[:].rearrange("p (j8 m) -> p j8 m", j8=J8)
        vstv = vst[:].rearrange("p (j8 b) -> p j8 b", j8=J8)
        for j4 in range(J4):
            nc.vector.tensor_copy(
                v0bdv[32 * j4:32 * j4 + 32, :, 32 * j4:32 * j4 + 32],
                vstv[32 * j4:32 * j4 + 32])

        # ---- Wv (PE -> PSUM), ACT evacuates, DVE mul+tree => A --------------
        # per j8: Wv[(j4,b), (k,i)]; A[b,j,k] = sum_i Wv*u
        A = pool.tile([128, J8 * KL], F32)
        Av = A[:].rearrange("p (j8 k) -> p j8 k", j8=J8)
        wvs = [pool.tile([128, KI], BF16, name=f"wvs{x}") for x in range(2)]
        CH = [(0, 512), (512, 512), (1024, 512), (1536, 512), (2048, 256)]
        exp_warmed = False
        for j8 in range(J8):
            wv = wvs[j8 % 2]
            for (c0, sz) in CH:
                pw = psum.tile([128, 512], F32, tag="wv", bufs=2)
                nc.tensor.matmul(pw[:, :sz], v0bdv[:, j8],
                                 wov[:, j8, c0:c0 + sz], start=True, stop=True)
                nc.scalar.activation(wv[:, c0:c0 + sz], pw[:, :sz], ACT.Copy)
            if not exp_warmed:
                # hide the Exp table load under the A-phase (ACT idles here)
                nc.scalar.activation(actwarm[:], actwarm[:], ACT.Exp)
                exp_warmed = True
            nc.vector.tensor_mul(wv[:], wv[:], urep[:])
            tv = wv[:].rearrange("p (k i) -> p k i", i=I)
            for w in (8, 4, 2):
                nc.vector.tensor_add(tv[:, :, 0:w], tv[:, :, 0:w], tv[:, :, w:2 * w])
            nc.vector.tensor_add(Av[:, j8], tv[:, :, 0], tv[:, :, 1])
        if stop_after == "A":
            return _finish(A[:], 128)

        # ---- c1 = softmax_j(A): no max-subtraction (|A| small) --------------
        ebf = pool.tile([128, J8 * KL], BF16)
        nc.scalar.activation(ebf[:], A[:], ACT.Exp)
        ebv = ebf[:].rearrange("p (j8 k) -> p j8 k", j8=J8)
        z1 = pool.tile([128, KL], F32)
        nc.vector.tensor_reduce(z1[:], ebv.transpose((0, 2, 1)), axis=AX, op=ADD)
        sh64 = pool.tile([64, KL], F32)
        nc.vector.tensor_copy(sh64[:], z1[64:128, :])
        z2 = pool.tile([64, KL], F32)
        nc.vector.tensor_add(z2[:], z1[0:64, :], sh64[:])
        sh32 = pool.tile([32, KL], F32)
        nc.vector.tensor_copy(sh32[:], z2[32:64, :])
        z3 = pool.tile([32, KL], F32)
        nc.vector.tensor_add(z3[:], z2[0:32, :], sh32[:])
        zr = pool.tile([32, KL], F32)
        nc.vector.reciprocal(zr[:], z3[:])
        zru = pool.tile([128, KL], BF16)
        nc.vector.tensor_copy(zru[0:32, :], zr[:])
        nc.vector.tensor_copy(zru[32:64, :], zru[0:32, :])
        nc.vector.tensor_copy(zru[64:128, :], zru[0:64, :])
        c1b = pool.tile([128, J8 * KL], BF16)
        zruv = zru[:].unsqueeze(1).broadcast_to((128, J8, KL))
        c1bv = c1b[:].rearrange("p (j8 k) -> p j8 k", j8=J8)
        nc.vector.tensor_mul(c1bv, ebv, zruv)
        if stop_after == "c1":
            return _finish(c1b[:], 128)

        # ---- s1 tail prep: c1 tail transposed + ih-replicated ---------------
        # transpose c1 tails -> [16=k16, (j8, j4, b)], then replicate over ih
        # via 8 small SBUF->SBUF DMAs (engines allow any partition base).
        pt2all = psum.tile([KT2, J8 * 128], BF16, tag="tr2")
        for j8 in range(J8):
            nc.tensor.matmul(pt2all[:KT2, 128 * j8:128 * j8 + 128],
                             c1bv[:, j8, KT1:KL], ident[:], is_transpose=True)
        c1k2s = pool.tile([KT2, J8 * 128], BF16)
        nc.vector.tensor_copy(c1k2s[:], pt2all[:KT2, :])
        c1k2p = pool.tile([128, J8 * 128], BF16)
        for ih in range(8):
            nc.sync.dma_start(c1k2p[16 * ih:16 * ih + 16, :], c1k2s[:])
        # cu2p[128=(ih,k16), (j8, il, j4, b)]
        cu2p = pool.tile([128, J8 * 2 * 128], BF16)
        cu2pv = cu2p[:].rearrange("p (j8 il m) -> p j8 il m", j8=J8, il=2)
        cu2pv5 = cu2p[:].rearrange("p (j8 il j4 b) -> p j8 il j4 b",
                                   j8=J8, il=2, j4=J4)
        c1k2p4 = c1k2p[:].rearrange("p (j8 j4 b) -> p j8 j4 b", j8=J8, j4=J4)
        u1bpv2 = u1bp[:].rearrange("p (il b) -> p il b", il=2)
        for il in range(2):
            u1bpb = (u1bpv2[:, il].unsqueeze(1).unsqueeze(1)
                     .broadcast_to((128, J8, J4, B)))
            nc.vector.tensor_mul(cu2pv5[:, :, il], c1k2p4, u1bpb)

        # ---- s1 per-j8 pipeline: transpose c1 -> cu -> 18 accum matmuls -----
        ps1full = psum.tile([128, JO], F32, tag="acc")
        ps1 = ps1full
        w1am = w1a[:].rearrange("k (i j8 m) -> k i j8 m", i=I, j8=J8)
        w1bpm = w1bp[:].rearrange("p (il j8 m) -> p il j8 m", il=2, j8=J8)
        u1ab = u1av.unsqueeze(2).broadcast_to((KT1, I, J4, B))
        for j8 in range(J8):
            ptc = psum.tile([128, 128], BF16, tag="tr", bufs=2)
            nc.tensor.matmul(ptc[:], c1bv[:, j8, 0:KT1], ident[:],
                             is_transpose=True)
            c1k1 = pool.tile([KT1, 128], BF16, tag="c1k1", bufs=2,
                             name=f"c1k1_{j8}")
            nc.scalar.activation(c1k1[:], ptc[:], ACT.Copy)
            cu1 = pool.tile([KT1, I * 128], BF16, tag="cu1", bufs=2,
                            name=f"cu1_{j8}")
            cu1v = cu1[:].rearrange("k (i m) -> k i m", i=I)
            cu1v4 = cu1[:].rearrange("k (i j4 b) -> k i j4 b", i=I, j4=J4)
            c1k1b = (c1k1[:].rearrange("k (j4 b) -> k j4 b", j4=J4)
                     .unsqueeze(1).broadcast_to((KT1, I, J4, B)))
            nc.vector.tensor_mul(cu1v4, c1k1b, u1ab)
            sl = slice(128 * j8, 128 * j8 + 128)
            for i in range(I):
                nc.tensor.matmul(ps1[:, sl], cu1v[:, i], w1am[:, i, j8],
                                 start=(i == 0), stop=False)
            for il in range(2):
                nc.tensor.matmul(ps1[:, sl], cu2pv[:, j8, il],
                                 w1bpm[:, il, j8], start=False, stop=(il == 1))

        # diagonal extraction: s1q[(j4,b), (j8,o)]
        s1q = pool.tile([128, J8 * O], F32)
        ps1v = ps1[:].rearrange("p (j8 j4 o) -> p j8 j4 o", j8=J8, j4=J4)
        s1qv = s1q[:].rearrange("p (j8 o) -> p j8 o", j8=J8)
        for j4 in range(J4):
            nc.vector.tensor_copy(s1qv[32 * j4:32 * j4 + 32],
                                  ps1v[32 * j4:32 * j4 + 32, :, j4])
        if stop_after == "s1":
            return _finish(s1q[:], 128)

        if use_cc:
            nc.gpsimd.dma_start(cc1_in[:], s1q[:])
            nc.gpsimd.collective_compute(
                "AllReduce", ADD, replica_groups=rg,
                ins=[cc1_in.opt()], outs=[cc1_out.opt()])
            s1g = pool.tile([128, J8 * O], F32)
            nc.gpsimd.dma_start(s1g[:], cc1_out[:])
        else:
            s1g = s1q

        # ---- out = squash(s1 + bias) ----------------------------------------
        s1f = pool.tile([128, J8 * O], F32)
        nc.vector.tensor_add(s1f[:], s1g[:], bias1[:])
        v1 = pool.tile([128, J8 * O], F32)
        squash(v1[:], s1f[:], J8)
        nc.scalar.dma_start(out_d, v1[:])


# ---------------------------------------------------------------------------
# compile + run
# ---------------------------------------------------------------------------

_CACHE = {}


def _get_compiled(use_cc=True, n_cores=NC, stop_after=None):
    key = (use_cc, n_cores, stop_after)
    if key in _CACHE:
        return _CACHE[key]
    import concourse.bacc as bacc
    import concourse.tile as tile
    from concourse import mybir

    nc = bacc.Bacc("TRN2", target_bir_lowering=False, debug=False,
                   num_devices=n_cores)
    F32 = mybir.dt.float32
    BF16 = mybir.dt.bfloat16
    shapes = {
        "w1a": ([KT1, I * J * O], BF16),
        "w1bp": ([128, 2 * J * O], BF16),
        "wo": ([J4 * O, J8 * KL * I], BF16),
        "u1a": ([KT1, I * B], BF16),
        "u1bp": ([128, 2 * B], BF16),
        "urep": ([J4 * B, KL * I], BF16),
        "bias0q": ([128, 256], F32),
        "bias1": ([J4 * B, J8 * O], F32),
    }
    ins = {k: nc.dram_tensor(k, sh, dt, kind="ExternalInput").ap()
           for k, (sh, dt) in shapes.items()}
    outs = {"out": nc.dram_tensor("out", [J4 * B, J8 * O], F32,
                                  kind="ExternalOutput").ap()}
    with tile.TileContext(nc) as tc:
        build_program(tc, outs, ins, n_cores=n_cores, use_cc=use_cc,
                      stop_after=stop_after)
    nc.compile()
    _CACHE[key] = nc
    return nc


def kernel(**inputs):
    from concourse import bass_utils

    in_maps = host_prep(inputs["u"], inputs["W"], inputs["bias"])
    nc = _get_compiled()
    res = bass_utils.run_bass_kernel_spmd(nc, in_maps, core_ids=list(range(NC)))
    return host_unpack(np.asarray(res.results[0]["out"], dtype=np.float32))
